# revision 1
# baseline (speedup 1.0000x reference)
"""Trainium2 Bass kernel for nn_DGRecLayer (greedy submodular neighbor
selection / DGRec message passing), 8-core SPMD.

Strategy (dst-sharded per the spec hint):
  - 8192 dst nodes sharded 1024/core; sims_table + h_src replicated.
  - Per core, per "superchunk" of 128 dsts: the 6400 (dst, i) pairs are
    processed 128 per pass.  Each pass fetches 128 table rows
    (per-partition-singleton indirect DMA, 64KB payload/partition), then a
    GPSIMD ap_gather extracts the needed columns (each 16-partition group's
    shared index list holds the pass's <=4 dsts' column lists; a partition
    slices out its own dst's 50 values).  Small SBUF->SBUF DMAs transpose
    the extracted rows into a dst-per-partition sims tile [128, 50, 50].
    Mail rows (degree-normalized h_src) are fetched the same way.
  - The 10-step greedy facility-location argmax runs on the DVE in the
    dst-per-partition layout; output = selection-weighted mail sum.

The host only preprocesses the int index tensor into DMA offsets / gather
indices (and bincounts it for the degree normalization); all float data is
touched exclusively on-device.
"""
import sys

sys.path.insert(0, "/opt/trn_rl_repo")

import numpy as np

NUM_SRC = 16384
NUM_DST = 8192
N = 50
F = 64
K_SEL = 10
NCORES = 8
B_CORE = NUM_DST // NCORES  # 1024
SC = 128  # dsts per superchunk
NSC = B_CORE // SC  # 8
PAIRS = SC * N  # 6400 pairs per superchunk
NPASS = PAIRS // 128  # 50 passes per superchunk
NIDX_PAD = 208  # padded ap_gather idx count (<= 4 dsts * 50, mult of 16)

_COMPILED = {}


def _pass_segments(k):
    """Static pair->dst structure of pass k (same for every superchunk/core).

    Returns (dlist, segments): dlist = ordered distinct superchunk-local
    dsts in the pass; segments = (p0, p1, d, i0, slot) where partitions
    [p0, p1) hold rows [i0, i0 + p1 - p0) of dst d, and slot indexes dlist.
    """
    lo, hi = 128 * k, 128 * (k + 1)
    segs = []
    p = lo
    while p < hi:
        d = p // N
        nxt = min(hi, (d + 1) * N)
        segs.append((p - lo, nxt - lo, d, p % N))
        p = nxt
    dlist = [s[2] for s in segs]
    segments = [(p0, p1, d, i0, j) for j, (p0, p1, d, i0) in enumerate(segs)]
    return dlist, segments


def _build_program(nsc=NSC):
    import concourse.bass as bass
    import concourse.bacc as bacc
    import concourse.mybir as mybir
    import concourse.tile as tile

    P = 128
    nc = bacc.Bacc(None, target_bir_lowering=False)
    f32, i32, i16 = mybir.dt.float32, mybir.dt.int32, mybir.dt.int16

    tab = nc.declare_dram_parameter("tab", [NUM_SRC, NUM_SRC], f32, isOutput=False)
    hsrc = nc.declare_dram_parameter("hsrc", [NUM_SRC, F], f32, isOutput=False)
    nrm = nc.declare_dram_parameter("nrm", [P, NUM_SRC // P], f32, isOutput=False)
    rowoff = nc.declare_dram_parameter("rowoff", [nsc, NPASS, P], i32, isOutput=False)
    mailoff = nc.declare_dram_parameter("mailoff", [nsc, NPASS, P], i32, isOutput=False)
    gidx = nc.declare_dram_parameter(
        "gidx", [nsc, NPASS, P, NIDX_PAD // 16], i16, isOutput=False
    )
    iot = nc.declare_dram_parameter("iot", [P, N], f32, isOutput=False)
    iotb = nc.declare_dram_parameter("iotb", [P, N], f32, isOutput=False)
    out = nc.declare_dram_parameter("out", [nsc * SC, F], f32, isOutput=True)

    norm_in = float(np.float32(np.clip(np.float32(N), 1.0, None)) ** np.float32(-0.5))

    with tile.TileContext(nc) as tc:
        with tc.tile_pool(name="dram", bufs=1, space="DRAM") as dp:
            featd = dp.tile([NUM_SRC, F], f32)

            # ---- stage 1: feat = h_src * deg_norm -> DRAM  (own pool, freed) ----
            with tc.tile_pool(name="featp", bufs=1) as fp:
                t_h = fp.tile([P, NUM_SRC // P, F], f32)
                nc.sync.dma_start(t_h[:], hsrc[:].rearrange("(t p) f -> p t f", p=P))
                t_n = fp.tile([P, NUM_SRC // P], f32)
                nc.sync.dma_start(t_n[:], nrm[:])
                nc.vector.tensor_tensor(
                    out=t_h[:],
                    in0=t_h[:],
                    in1=t_n[:].rearrange("p (t o) -> p t o", o=1).to_broadcast(
                        [P, NUM_SRC // P, F]
                    ),
                    op=mybir.AluOpType.mult,
                )
                nc.sync.dma_start(featd[:].rearrange("(t p) f -> p t f", p=P), t_h[:])

            # ---- stage 2: main loop ----
            with (
                tc.tile_pool(name="big", bufs=2) as bigp,
                tc.tile_pool(name="simsp", bufs=1) as simsp,
                tc.tile_pool(name="small", bufs=2) as smallp,
                tc.tile_pool(name="work", bufs=1) as workp,
            ):
                t_iot = workp.tile([P, N], f32, tag="iot")
                nc.sync.dma_start(t_iot[:], iot[:])
                t_iotb = workp.tile([P, N], f32, tag="iotb")
                nc.sync.dma_start(t_iotb[:], iotb[:])

                for s in range(nsc):
                    t_sims = simsp.tile([P, N, N], f32, tag="sims")
                    t_mail = simsp.tile([P, N, F], f32, tag="mail")
                    for k in range(NPASS):
                        dlist, segments = _pass_segments(k)
                        t_ro = smallp.tile([P, 1], i32, tag="ro")
                        nc.sync.dma_start(
                            t_ro[:], rowoff[s, k].rearrange("(p o) -> p o", o=1)
                        )
                        t_panel = bigp.tile([P, NUM_SRC], f32, tag="panel")
                        nc.gpsimd.indirect_dma_start(
                            out=t_panel[:],
                            out_offset=None,
                            in_=tab[:],
                            in_offset=bass.IndirectOffsetOnAxis(ap=t_ro[:], axis=0),
                        )
                        t_mo = smallp.tile([P, 1], i32, tag="mo")
                        nc.sync.dma_start(
                            t_mo[:], mailoff[s, k].rearrange("(p o) -> p o", o=1)
                        )
                        t_mrow = smallp.tile([P, F], f32, tag="mrow")
                        nc.gpsimd.indirect_dma_start(
                            out=t_mrow[:],
                            out_offset=None,
                            in_=featd[:],
                            in_offset=bass.IndirectOffsetOnAxis(ap=t_mo[:], axis=0),
                        )
                        t_gi = smallp.tile([P, NIDX_PAD // 16], i16, tag="gi")
                        nc.sync.dma_start(t_gi[:], gidx[s, k])
                        t_ext = smallp.tile([P, NIDX_PAD], f32, tag="ext")
                        nc.gpsimd.ap_gather(
                            out_ap=t_ext[:],
                            in_ap=t_panel[:],
                            idxs_ap=t_gi[:],
                            channels=P,
                            num_elems=NUM_SRC,
                            d=1,
                            num_idxs=NIDX_PAD,
                        )
                        for (p0, p1, d, i0, slot) in segments:
                            npair = p1 - p0
                            nc.sync.dma_start(
                                t_sims[d : d + 1, i0 : i0 + npair, :],
                                t_ext[p0:p1, slot * N : (slot + 1) * N],
                            )
                            nc.sync.dma_start(
                                t_mail[d : d + 1, i0 : i0 + npair, :],
                                t_mrow[p0:p1, :],
                            )

                    # ---- greedy on t_sims [128 dst, 50 i, 50 j] ----
                    t_K = workp.tile([P, N], f32, tag="K")
                    nc.vector.memset(t_K[:], 0.0)
                    t_wsel = workp.tile([P, N], f32, tag="wsel")
                    nc.vector.memset(t_wsel[:], 0.0)
                    t_tmp = workp.tile([P, N, N], f32, tag="tmp")
                    t_g = workp.tile([P, N], f32, tag="g")
                    t_sk = workp.tile([P, 1], f32, tag="sk")
                    t_m = workp.tile([P, 1], f32, tag="m")
                    t_eq = workp.tile([P, N], f32, tag="eq")
                    t_cand = workp.tile([P, N], f32, tag="cand")
                    t_fi = workp.tile([P, 1], f32, tag="fi")
                    t_first = workp.tile([P, N], f32, tag="first")
                    t_mask2 = workp.tile([P, N, N], f32, tag="mask2")

                    for _step in range(K_SEL):
                        nc.vector.tensor_tensor(
                            out=t_tmp[:],
                            in0=t_sims[:],
                            in1=t_K[:]
                            .rearrange("p (o j) -> p o j", o=1)
                            .to_broadcast([P, N, N]),
                            op=mybir.AluOpType.max,
                        )
                        nc.vector.tensor_reduce(
                            out=t_g[:],
                            in_=t_tmp[:],
                            axis=mybir.AxisListType.X,
                            op=mybir.AluOpType.add,
                        )
                        nc.vector.tensor_reduce(
                            out=t_sk[:],
                            in_=t_K[:].rearrange("p (o j) -> p o j", o=1),
                            axis=mybir.AxisListType.X,
                            op=mybir.AluOpType.add,
                        )
                        nc.vector.tensor_tensor(
                            out=t_g[:],
                            in0=t_g[:],
                            in1=t_sk[:].to_broadcast([P, N]),
                            op=mybir.AluOpType.subtract,
                        )
                        nc.vector.tensor_reduce(
                            out=t_m[:],
                            in_=t_g[:].rearrange("p (o j) -> p o j", o=1),
                            axis=mybir.AxisListType.X,
                            op=mybir.AluOpType.max,
                        )
                        nc.vector.tensor_tensor(
                            out=t_eq[:],
                            in0=t_g[:],
                            in1=t_m[:].to_broadcast([P, N]),
                            op=mybir.AluOpType.is_equal,
                        )
                        nc.vector.scalar_tensor_tensor(
                            out=t_cand[:],
                            in0=t_eq[:],
                            scalar=-1024.0,
                            in1=t_iotb[:],
                            op0=mybir.AluOpType.mult,
                            op1=mybir.AluOpType.add,
                        )
                        nc.vector.tensor_reduce(
                            out=t_fi[:],
                            in_=t_cand[:].rearrange("p (o j) -> p o j", o=1),
                            axis=mybir.AxisListType.X,
                            op=mybir.AluOpType.min,
                        )
                        nc.vector.scalar_tensor_tensor(
                            out=t_first[:],
                            in0=t_fi[:].to_broadcast([P, N]),
                            scalar=1.0,
                            in1=t_iot[:],
                            op0=mybir.AluOpType.mult,
                            op1=mybir.AluOpType.is_equal,
                        )
                        nc.vector.tensor_tensor(
                            out=t_wsel[:],
                            in0=t_wsel[:],
                            in1=t_first[:],
                            op=mybir.AluOpType.add,
                        )
                        # K_j = max_i first_i * tmp_ij  (j-major contiguous out)
                        nc.vector.tensor_tensor(
                            out=t_mask2[:],
                            in0=t_tmp[:].rearrange("p i j -> p j i"),
                            in1=t_first[:]
                            .rearrange("p (o i) -> p o i", o=1)
                            .to_broadcast([P, N, N]),
                            op=mybir.AluOpType.mult,
                        )
                        nc.vector.tensor_reduce(
                            out=t_K[:],
                            in_=t_mask2[:],
                            axis=mybir.AxisListType.X,
                            op=mybir.AluOpType.max,
                        )

                    # ---- out = norm_in * sum_i wsel_i * mail_i ----
                    t_wm = workp.tile([P, F, N], f32, tag="wm")
                    nc.vector.tensor_tensor(
                        out=t_wm[:],
                        in0=t_mail[:].rearrange("p i f -> p f i"),
                        in1=t_wsel[:]
                        .rearrange("p (o i) -> p o i", o=1)
                        .to_broadcast([P, F, N]),
                        op=mybir.AluOpType.mult,
                    )
                    t_out = workp.tile([P, F], f32, tag="outt")
                    nc.vector.tensor_reduce(
                        out=t_out[:],
                        in_=t_wm[:],
                        axis=mybir.AxisListType.X,
                        op=mybir.AluOpType.add,
                    )
                    nc.vector.tensor_scalar_mul(t_out[:], t_out[:], norm_in)
                    nc.sync.dma_start(out[s * SC : (s + 1) * SC, :], t_out[:])

    nc.finalize()
    return nc


def _prepare_core_inputs(h_src, sims_table, neighbor_idx, core, nsc=NSC):
    """Host-side index preprocessing for one core (float data untouched)."""
    nbr = neighbor_idx  # int64 [NUM_DST, N]
    flat = nbr.reshape(-1)
    deg = np.bincount(flat, minlength=NUM_SRC).astype(np.float32)
    norm = np.clip(deg, 1.0, None).astype(np.float32) ** np.float32(-0.5)
    nrm = np.ascontiguousarray(norm.reshape(NUM_SRC // 128, 128).T)

    cn = nbr[core * B_CORE : (core + 1) * B_CORE].astype(np.int64)  # [1024, 50]
    rowoff = np.zeros((nsc, NPASS, 128), np.int32)
    mailoff = np.zeros((nsc, NPASS, 128), np.int32)
    gidx = np.zeros((nsc, NPASS, 128, NIDX_PAD // 16), np.int16)
    wrap_rows = np.arange(NIDX_PAD) % 16
    wrap_cols = np.arange(NIDX_PAD) // 16
    for s in range(nsc):
        sn = cn[s * SC : (s + 1) * SC]  # [128, 50]
        q = np.arange(PAIRS)
        r = sn[q // N, q % N]
        rowoff[s] = r.astype(np.int32).reshape(NPASS, 128)
        mailoff[s] = r.astype(np.int32).reshape(NPASS, 128)
        for k in range(NPASS):
            dlist, _segs = _pass_segments(k)
            idxlist = np.zeros(NIDX_PAD, np.int16)
            for j, dl in enumerate(dlist):
                idxlist[j * N : (j + 1) * N] = sn[dl].astype(np.int16)
            w = np.zeros((16, NIDX_PAD // 16), np.int16)
            w[wrap_rows, wrap_cols] = idxlist
            gidx[s, k] = np.tile(w, (8, 1))
    iot = np.broadcast_to(np.arange(N, dtype=np.float32), (128, N)).copy()
    iotb = np.broadcast_to(np.arange(N, dtype=np.float32) + 1024.0, (128, N)).copy()
    return {
        "tab": sims_table,
        "hsrc": h_src,
        "nrm": nrm,
        "rowoff": rowoff,
        "mailoff": mailoff,
        "gidx": gidx,
        "iot": iot,
        "iotb": iotb,
    }


def _run_spmd(nc, in_maps, n_timed=0):
    import jax
    import concourse.mybir as mybir
    from jax.sharding import Mesh, PartitionSpec, NamedSharding
    from jax.experimental.shard_map import shard_map
    from concourse import bass2jax

    bass2jax.install_neuronx_cc_hook()
    partition_name = nc.partition_id_tensor.name if nc.partition_id_tensor else None
    in_names, out_names, out_avals, zero_outs = [], [], [], []
    for alloc in nc.m.functions[0].allocations:
        if not isinstance(alloc, mybir.MemoryLocationSet):
            continue
        name = alloc.memorylocations[0].name
        if alloc.kind == "ExternalInput":
            if name != partition_name:
                in_names.append(name)
        elif alloc.kind == "ExternalOutput":
            out_names.append(name)
            shape = tuple(alloc.tensor_shape)
            dtype = mybir.dt.np(alloc.dtype)
            out_avals.append(jax.core.ShapedArray(shape, dtype))
            zero_outs.append(np.zeros(shape, dtype))
    n_params = len(in_names)
    all_in_names = list(in_names) + list(out_names)
    if partition_name is not None:
        all_in_names.append(partition_name)

    def _body(*args):
        operands = list(args)
        if partition_name is not None:
            operands.append(bass2jax.partition_id_tensor())
        outs = bass2jax._bass_exec_p.bind(
            *operands,
            out_avals=tuple(out_avals),
            in_names=tuple(all_in_names),
            out_names=tuple(out_names),
            lowering_input_output_aliases=(),
            sim_require_finite=True,
            sim_require_nnan=True,
            nc=nc,
        )
        return tuple(outs)

    n_cores = len(in_maps)
    devices = jax.devices()[:n_cores]
    mesh = Mesh(np.asarray(devices), ("core",))
    n_in = n_params + len(out_names)
    sharded = jax.jit(
        shard_map(
            _body,
            mesh=mesh,
            in_specs=(PartitionSpec("core"),) * n_in,
            out_specs=(PartitionSpec("core"),) * len(out_names),
            check_rep=False,
        ),
        keep_unused=True,
    )
    sh = NamedSharding(mesh, PartitionSpec("core"))
    dev_in = [
        jax.device_put(
            np.concatenate([np.asarray(m[name]) for m in in_maps], axis=0), sh
        )
        for name in in_names
    ] + [jax.device_put(np.concatenate([z] * n_cores, axis=0), sh) for z in zero_outs]
    out_arrs = sharded(*dev_in)
    jax.block_until_ready(out_arrs)
    times = []
    if n_timed:
        import time

        for _ in range(n_timed):
            t0 = time.perf_counter()
            out_arrs = sharded(*dev_in)
            jax.block_until_ready(out_arrs)
            times.append(time.perf_counter() - t0)
    results = [
        {
            name: np.asarray(out_arrs[i]).reshape(n_cores, *out_avals[i].shape)[c]
            for i, name in enumerate(out_names)
        }
        for c in range(n_cores)
    ]
    return results, times


def kernel(h_src, sims_table, neighbor_idx, _timing=None):
    h_src = np.ascontiguousarray(np.asarray(h_src, dtype=np.float32))
    sims_table = np.ascontiguousarray(np.asarray(sims_table, dtype=np.float32))
    nbr = np.asarray(neighbor_idx).astype(np.int64)

    if "nc" not in _COMPILED:
        _COMPILED["nc"] = _build_program()
    nc = _COMPILED["nc"]

    in_maps = [_prepare_core_inputs(h_src, sims_table, nbr, c) for c in range(NCORES)]
    results, times = _run_spmd(nc, in_maps, n_timed=(_timing or 0))
    if _timing:
        _COMPILED["times"] = times
    out = np.concatenate([results[c]["out"] for c in range(NCORES)], axis=0)
    return out.astype(np.float32)

